# revision 6
# baseline (speedup 1.0000x reference)
# Trainium2 Bass kernel for nn_DebiasedRNN (GRU-like attention-gated RNN over
# packed sequences).  Contract: kernel(**inputs) takes the FULL unsharded
# inputs (numpy) and returns the FULL [T, B, H] float32 output.
#
# Strategy (v4)
# -------------
# Data-parallel over batch: 8 NeuronCores x 32 rows each.  All sequence
# masking is folded into the attention scores on the host (a_t := 0 for
# t >= length makes the recurrence carry h exactly; masked outputs are
# re-zeroed on the host), so the device program is input-independent and
# identical on every core (true SPMD).
#
# The kernel is bound by the per-step dependency chain
#
#   pn(t-1) -> PE: W_rh@{gq,pn} -> Act: sigmoid(r) -> DVE: r*h ->
#   PE: W_hh@rh -> Act: tanh -> DVE: pn
#
# v3 latency engineering:
#  * ONE PSUM tile per gate PER STEP (r/u/h): no intra-chunk reader chains,
#    so every chain instruction has a single PE producer wait that the Tile
#    scheduler keeps inline (pre-decoded, engine-level wait) instead of a
#    SEQ-blocking standalone semaphore op.
#  * gate biases ride the activation instructions as per-partition bias APs
#    (f32), so there are no bias matmuls at all.
#  * per gate, the gq matmul is issued before the pn one: gq is ready early,
#    so the gate's semaphore fires right after the single pn matmul drains.
#  * ALL elementwise consumers of h (r*h, gq, pn, hnew) live on the DVE, so
#    the h handoff between steps is same-engine program order.
#  * x-projections for step t+1 run on the PE during step t's idle window.
#
# The host does every layout change (shard / transpose / bf16-cast /
# output transpose + masking) in numpy.

import numpy as np
import ml_dtypes

import concourse.bass as bass
import concourse.tile as tile
from concourse import bacc, mybir
from concourse.bass_utils import run_bass_kernel_spmd

T, B, D, H = 512, 256, 128, 128
NCORES = 8
BS = B // NCORES            # 32 batch rows per core
NCOLS = T * BS              # 16384 (t, b) columns per core
OUTCH = 32                  # steps per output staging chunk (1 MB DMA)

F32 = mybir.dt.float32
BF16 = mybir.dt.bfloat16
AF = mybir.ActivationFunctionType
OP = mybir.AluOpType

_BF = ml_dtypes.bfloat16


def build_nc(t_steps=T, opts=()):
    """Build the (input-independent) single-core Bass program."""
    nc = bacc.Bacc("TRN2")

    # ---- DRAM I/O ---------------------------------------------------------
    xT = nc.dram_tensor("xT", [128, NCOLS], BF16, kind="ExternalInput")
    attr = nc.dram_tensor("attr", [1, NCOLS], BF16, kind="ExternalInput")
    # all six 128x128 weights packed side by side: one startup DMA
    wall = nc.dram_tensor("wall", [128, 6 * 128], BF16, kind="ExternalInput")
    # bias columns (0:3) + host-precomputed step-0 projections (3:):
    # one tensor, one early DMA descriptor
    b3z0 = nc.dram_tensor("b3z0", [128, 3 + 3 * BS], F32,
                          kind="ExternalInput")
    outT = nc.dram_tensor("outT", [128, NCOLS], BF16, kind="ExternalOutput")

    with tile.TileContext(nc) as tc:
        with (
            tc.tile_pool(name="const", bufs=1) as const,
            tc.tile_pool(name="stage_p", bufs=3) as stage_p,
            tc.tile_pool(name="work", bufs=4) as work,
            tc.tile_pool(name="r_pool", bufs=2, space="PSUM") as r_pool,
            tc.tile_pool(name="u_pool", bufs=2, space="PSUM") as u_pool,
            tc.tile_pool(name="h_pool", bufs=2, space="PSUM") as h_pool,
        ):
            # ---- constants / resident inputs ------------------------------
            # weights + biases first: the first steps need them, while x/att
            # slices stream in behind them.
            wall_sb = const.tile([128, 6 * 128], BF16, name="wall_sb")
            b3z0_sb = const.tile([128, 3 + 3 * BS], F32, name="b3z0_sb")
            nc.sync.dma_start(out=b3z0_sb[:], in_=b3z0[:])
            nc.sync.dma_start(out=wall_sb[:], in_=wall[:])
            b3_st = b3z0_sb[:, 0:3]
            z0_sb = b3z0_sb[:, 3:]
            w_rh_sb = wall_sb[:, 0 * 128:1 * 128]
            w_uh_sb = wall_sb[:, 1 * 128:2 * 128]
            w_hh_sb = wall_sb[:, 2 * 128:3 * 128]
            w_rx_sb = wall_sb[:, 3 * 128:4 * 128]
            w_ux_sb = wall_sb[:, 4 * 128:5 * 128]
            w_hx_sb = wall_sb[:, 5 * 128:6 * 128]

            xT_sb = const.tile([128, NCOLS], BF16, name="xT_sb")
            att_sb = const.tile([128, NCOLS], BF16, name="att_sb")
            NSL = 16
            for j in range(NSL):
                sl = slice(j * (NCOLS // NSL), (j + 1) * (NCOLS // NSL))
                nc.sync.dma_start(out=xT_sb[:, sl], in_=xT[:, sl])
                # broadcast the attention row across all 128 partitions
                att_bc = bass.AP(
                    tensor=attr,
                    offset=j * (NCOLS // NSL),
                    ap=[[0, 128], [1, NCOLS // NSL]],
                )
                nc.gpsimd.dma_start(out=att_sb[:, sl], in_=att_bc)

            h0_f = const.tile([128, BS], BF16, name="h0_f")
            nc.vector.memset(h0_f[:], 0.0)
            # per-step bias tiles are re-written by the DVE (below); the DVE
            # sits on the chain's transitive path, so the activations' bias
            # deps stay cheap.  b3_cur covers step t, b3_nxt step t+1.
            # dummy sigmoid first: forces the act-table set that contains
            # sigmoid+tanh+copy to load ONCE, early, under the DMA shadow
            # (otherwise the Copy below loads a copy-only set and the first
            # real sigmoid pays a second 1.3us table load on the start path)
            warm = const.tile([1, 1], F32, name="warm")
            nc.vector.memset(warm[:], 0.0)
            warm2 = const.tile([1, 1], F32, name="warm2")
            nc.scalar.activation(warm2[:], warm[:], AF.Sigmoid)
            b3_cur = work.tile([128, 3], F32, name="b3c", tag="b3")
            nc.scalar.activation(b3_cur[:], b3_st, AF.Copy)

            hp_f = h0_f[:]     # h_{t-1} (f32)
            pp = None          # pn_{t-1} (bf16)  [chain state]
            gq = None          # gq_{t-1} (bf16)
            r_ps = [None, None]
            u_ps = [None, None]
            h_ps = [None, None]
            stage = None
            xv = xT_sb.rearrange("p (t b) -> p t b", b=BS)

            def preamble(t):
                """x-projections for step t into fresh per-step PSUM tiles."""
                i = t % 2
                xsl = xv[:, t:t + 1, :]
                r_ps[i] = r_pool.tile([128, BS], F32, name="r_ps",
                                      tag=f"r{i}", bufs=1)
                u_ps[i] = u_pool.tile([128, BS], F32, name="u_ps",
                                      tag=f"u{i}", bufs=1)
                h_ps[i] = h_pool.tile([128, BS], F32, name="h_ps",
                                      tag=f"h{i}", bufs=1)
                nc.tensor.matmul(r_ps[i][:, :], w_rx_sb, xsl,
                                 start=True, stop=False,
                                 skip_group_check=True)
                nc.tensor.matmul(u_ps[i][:, :], w_ux_sb, xsl,
                                 start=True, stop=False,
                                 skip_group_check=True)
                nc.tensor.matmul(h_ps[i][:, :], w_hx_sb, xsl,
                                 start=True, stop=False,
                                 skip_group_check=True)

            for t in range(t_steps):
                tm = t % T
                i = t % 2
                if t % OUTCH == 0:
                    stage = stage_p.tile([128, OUTCH * BS], BF16, name="stage",
                                         tag="stage")
                off = (t % OUTCH) * BS

                # -- recurrent matmuls: h(t-1) enters as pn + gq ------------
                # gq is ready early, so issue it first; the gate semaphore
                # then fires right after the single pn matmul drains.
                if t > 0:
                    nc.tensor.matmul(r_ps[i][:, :], w_rh_sb, gq[:],
                                     start=False, stop=False,
                                     skip_group_check=True)
                    nc.tensor.matmul(r_ps[i][:, :], w_rh_sb, pp[:],
                                     start=False, stop=True,
                                     skip_group_check=True)
                    nc.tensor.matmul(u_ps[i][:, :], w_uh_sb, gq[:],
                                     start=False, stop=False,
                                     skip_group_check=True)
                    nc.tensor.matmul(u_ps[i][:, :], w_uh_sb, pp[:],
                                     start=False, stop=True,
                                     skip_group_check=True)


                # x-projections for step t+1 (PE idle window)
                if t + 1 < t_steps:
                    preamble(t + 1)

                # -- gates (Act); biases ride as per-partition APs ----------
                r_sb = work.tile([128, BS], BF16, name="r_sb", tag="r_sb")
                u_sb = work.tile([128, BS], BF16, name="u_sb", tag="u_sb")
                z_r = r_ps[i][:, :] if t > 0 else z0_sb[:, 0:BS]
                z_u = u_ps[i][:, :] if t > 0 else z0_sb[:, BS:2 * BS]
                nc.scalar.activation(r_sb[:], z_r, AF.Sigmoid,
                                     bias=b3_cur[:, 0:1])
                nc.scalar.activation(u_sb[:], z_u, AF.Sigmoid,
                                     bias=b3_cur[:, 1:2])

                # -- chain: rh = r * h_{t-1} (DVE), then W_hh matmul --------
                rh = work.tile([128, BS], BF16, name="rh", tag="rh")
                nc.vector.tensor_mul(rh[:], r_sb[:], hp_f)
                if t > 0:
                    nc.tensor.matmul(h_ps[i][:, :], w_hh_sb, rh[:],
                                     start=False, stop=True,
                                     skip_group_check=True)

                # off-chain (DVE): ua = u * att; gq = h - ua*h
                ua = work.tile([128, BS], BF16, name="ua", tag="ua")
                nc.vector.tensor_mul(ua[:], u_sb[:],
                                     att_sb[:, tm * BS:(tm + 1) * BS])
                uah = work.tile([128, BS], BF16, name="uah", tag="uah")
                nc.vector.tensor_mul(uah[:], ua[:], hp_f)
                gq_n = work.tile([128, BS], BF16, name="gq_n", tag="gq_n")
                nc.vector.tensor_sub(gq_n[:], hp_f, uah[:])

                that = work.tile([128, BS], BF16, name="that", tag="that")
                z_h = h_ps[i][:, :] if t > 0 else z0_sb[:, 2 * BS:3 * BS]
                nc.scalar.activation(that[:], z_h, AF.Tanh,
                                     bias=b3_cur[:, 2:3])

                # fresh bias tile for step t+1, written on the Act engine in
                # its idle window: the next step's activations then see a
                # recent same-engine producer (nosync dep), keeping their
                # single inline wait slot for the PE matmul.
                if t + 1 < t_steps:
                    b3_nxt = work.tile([128, 3], F32, name="b3n", tag="b3")
                    nc.scalar.activation(b3_nxt[:], b3_st, AF.Copy)

                # chain tail: pn = ua * tanh (plain tensor_mul -> DVE 2x)
                pn = work.tile([128, BS], BF16, name="pn", tag="pn")
                nc.vector.tensor_mul(pn[:], ua[:], that[:])

                # h(t) = pn + gq on DVE: all h consumers are DVE, so the
                # handoff to step t+1 is same-engine program order.
                hnew = stage[:, off:off + BS]
                nc.vector.tensor_add(hnew, pn[:], gq_n[:])

                hp_f = hnew
                pp = pn
                gq = gq_n
                if t + 1 < t_steps:
                    b3_cur = b3_nxt

                last_stage = ((t_steps - 1) // OUTCH) * OUTCH
                if t < last_stage:
                    if t % OUTCH == OUTCH - 1:
                        ob = (tm - (OUTCH - 1)) * BS
                        nc.sync.dma_start(out=outT[:, ob:ob + OUTCH * BS],
                                          in_=stage[:])
                elif (t - last_stage) % 8 == 7 or t == t_steps - 1:
                    # final stage buffer drains in 8-step slices as the steps
                    # complete, so the kernel doesn't end on one large DMA
                    g0 = ((t - last_stage) // 8) * 8
                    n = t - last_stage - g0 + 1
                    ob = (last_stage + g0) * BS
                    nc.sync.dma_start(out=outT[:, ob:ob + n * BS],
                                      in_=stage[:, g0 * BS:(g0 + n) * BS])
    nc.compile()
    return nc


_NC_CACHE = None


def _get_nc():
    global _NC_CACHE
    if _NC_CACHE is None:
        _NC_CACHE = build_nc()
    return _NC_CACHE


def prep_in_maps(inputs, att_scores, lengths, W_r, b_r, W_u, b_u, W_h, b_h):
    """Host-side shard + layout prep.  Returns per-core input dicts."""
    x = np.asarray(inputs, np.float32)
    att = np.asarray(att_scores, np.float32)
    lens = np.asarray(lengths, np.int64)
    mask = np.arange(T)[:, None] < lens[None, :]          # [T, B]
    # fold the masking into the attention scores (positive convention:
    # pn = ua * tanh and gq = h - ua*h are plain tensor-tensor ops)
    att_m = np.where(mask, att, 0.0).astype(np.float32)

    wall = np.concatenate([W_r[D:, :], W_u[D:, :], W_h[D:, :],
                           W_r[:D, :], W_u[:D, :], W_h[:D, :]],
                          axis=1)
    shared = dict(
        wall=np.ascontiguousarray(wall).astype(_BF),
    )
    b3c = np.stack([np.asarray(b_r, np.float32),
                    np.asarray(b_u, np.float32),
                    np.asarray(b_h, np.float32)], axis=1)

    in_maps = []
    for k in range(NCORES):
        bs = slice(k * BS, (k + 1) * BS)
        xk = x[:, bs, :]                                   # [T, 32, 128]
        xTk = np.ascontiguousarray(xk.transpose(2, 0, 1)).reshape(128, NCOLS)
        attk = np.ascontiguousarray(att_m[:, bs]).reshape(1, NCOLS).astype(_BF)
        # step-0 projections (h0 = 0): z = x0 @ W_{*x}, matching the device
        # path's bf16 rounding of x and W
        x0b = xk[0].astype(_BF).astype(np.float32)         # [32, 128]
        z0k = np.concatenate(
            [(x0b @ W_r[:D, :].astype(_BF).astype(np.float32)).T,
             (x0b @ W_u[:D, :].astype(_BF).astype(np.float32)).T,
             (x0b @ W_h[:D, :].astype(_BF).astype(np.float32)).T],
            axis=1).astype(np.float32)                     # [128, 3*BS]
        m = dict(shared)
        m["xT"] = xTk.astype(_BF)
        m["attr"] = attk
        m["b3z0"] = np.ascontiguousarray(
            np.concatenate([b3c, z0k], axis=1))
        in_maps.append(m)
    return in_maps, mask


def assemble_output(core_outs, mask):
    """[128, T*BS] per core -> [T, B, H] with masked rows zeroed."""
    parts = []
    for r in core_outs:
        o = np.asarray(r["outT"]).astype(np.float32).reshape(128, T, BS)
        parts.append(o.transpose(1, 2, 0))                 # [T, 32, 128]
    full = np.concatenate(parts, axis=1)                   # [T, B, H]
    return np.where(mask[:, :, None], full, 0.0).astype(np.float32)


def kernel(inputs, att_scores, lengths, W_r, b_r, W_u, b_u, W_h, b_h):
    nc = _get_nc()
    in_maps, mask = prep_in_maps(inputs, att_scores, lengths,
                                 W_r, b_r, W_u, b_u, W_h, b_h)
    res = run_bass_kernel_spmd(nc, in_maps, core_ids=list(range(NCORES)))
    return assemble_output(res.results, mask)
